# revision 1
# baseline (speedup 1.0000x reference)
"""GQA forward on 8 Trainium2 NeuronCores — v2.

Sharding: core c -> batch b=c//4, kv-head pair p=c%4 (kv heads {2p,2p+1},
q heads 8p..8p+7). Each core computes a partial [T,E] output (its heads'
contribution through Wo rows); host sums the 4 partials per batch + bo.

v2 changes over baseline:
- Stage A (QKV proj): x DMA'd in [128,512] column blocks so compute starts
  ~7us in; PE warm-up matmul chain rides out the DMA gate (HAM clock).
  Rope in bf16 on DVE (2x mode), Q/K cast to bf16 on ScalarE.
- Stage B+C fused per query-block isc: attention and the output projection
  interleave so the PE never drains at phase boundaries.
- A@V in bf16 with the [V|ones] ones-trick for the softmax denominator Z
  (fp8 V quantization costs ~3.6% broad error -> fails the 2e-2 gate).
- Finalize copy-defers psOT to SBUF (frees the PSUM bank in ~1.3us so the
  next head-pair's A@V proceeds); the slow DVE reciprocal + normalize run
  with a full head-pair of slack. Stage C for block isc is emitted inside
  block isc+1 so the last finalize is off the critical path.
- Causal masks on GpSimd; rope tables DMA'd upfront (sync queue is serial).
"""
import sys
import numpy as np

sys.path.insert(0, "/opt/trn_rl_repo")

import ml_dtypes

BF16 = ml_dtypes.bfloat16
FP8 = ml_dtypes.float8_e4m3fn

B, T, E = 2, 2048, 2048
HQ, HKV = 32, 8
D = 64
NT = T // 128          # 16 t-chunks
KC = 17                # augmented contraction chunks (2048 + bias row -> 2176)
KAUG = KC * 128
NWARM = 220            # PE warm-up matmuls riding out the initial DMA gate

_cache = {}


def _build_program():
    import concourse.bass as bass
    import concourse.tile as tile
    import concourse.mybir as mybir
    from concourse import bacc

    fp32 = mybir.dt.float32
    bf16 = mybir.dt.bfloat16
    f8 = mybir.dt.float8e4
    MUL = mybir.AluOpType.mult
    ADD = mybir.AluOpType.add
    SUB = mybir.AluOpType.subtract
    EXP = mybir.ActivationFunctionType.Exp

    nc = bacc.Bacc("TRN2", target_bir_lowering=False, debug=False)

    xt_d = nc.dram_tensor("xt", [KAUG, T], bf16, kind="ExternalInput").ap()
    wq_d = nc.dram_tensor("wq", [KAUG, 512], bf16, kind="ExternalInput").ap()
    wkv_d = nc.dram_tensor("wkv", [KAUG, 256], bf16, kind="ExternalInput").ap()
    wo_d = nc.dram_tensor("wo", [512, T], bf16, kind="ExternalInput").ap()
    rope_d = nc.dram_tensor("rope", [T, 512], bf16, kind="ExternalInput").ap()
    mask_d = nc.dram_tensor("mask", [128, 128], bf16, kind="ExternalInput").ap()
    iden_d = nc.dram_tensor("iden", [128, 128], bf16, kind="ExternalInput").ap()
    out_d = nc.dram_tensor("out", [T, E], bf16, kind="ExternalOutput").ap()

    def hv(ap, H, off, w):
        # [128, H*64] -> [128, H, w] slice of each head's d-range [off, off+w)
        return ap.rearrange("p (h d) -> p h d", h=H)[:, :, off:off + w]

    with tile.TileContext(nc) as tc:
        with (
            tc.tile_pool(name="persist", bufs=1) as pp,
            tc.tile_pool(name="wpool", bufs=1) as wp,
        ):
            iden = pp.tile([128, 128], bf16)
            nc.sync.dma_start(iden[:], iden_d[:])
            mask = pp.tile([128, 128], bf16)
            nc.sync.dma_start(mask[:], mask_d[:])

            # weight tiles (persistent); DMA emitted inside stage A, kc-interleaved
            wqs = [wp.tile([128, 512], bf16, tag=f"wq{kc}", name=f"wq{kc}")
                   for kc in range(KC)]
            wkvs = [wp.tile([128, 256], bf16, tag=f"wkv{kc}", name=f"wkv{kc}")
                    for kc in range(KC)]

            # persistent intermediates
            QT = [pp.tile([128, T], bf16, tag=f"QT{i}", name=f"QT{i}") for i in range(4)]
            KTd = [pp.tile([128, T], bf16, tag=f"KTd{i}", name=f"KTd{i}") for i in range(2)]
            yT = [pp.tile([128, T], bf16, tag=f"yT{i}", name=f"yT{i}") for i in range(4)]
            Vs = [pp.tile([128, 256], bf16, tag=f"V{j}", name=f"V{j}") for j in range(NT)]

            # ---------------- Stage A: projections + rope + transposes --------
            with (
                tc.tile_pool(name="psA", bufs=2, space="PSUM") as psA,
                tc.tile_pool(name="pst", bufs=2, space="PSUM") as pst,
                tc.tile_pool(name="psW", bufs=1, space="PSUM") as psWp,
                tc.tile_pool(name="xpool", bufs=1) as xp,
                tc.tile_pool(name="tmp", bufs=2) as tp,
                tc.tile_pool(name="qk", bufs=2) as qkp,
            ):
                # x column blocks + rope tables (stage-A lifetime only)
                xtt = [[None] * 4 for _ in range(KC)]
                for kc in range(KC):
                    nc.sync.dma_start(wqs[kc][:], wq_d[kc * 128:(kc + 1) * 128, :])
                    nc.sync.dma_start(wkvs[kc][:], wkv_d[kc * 128:(kc + 1) * 128, :])
                    xt0 = xp.tile([128, 512], bf16, tag=f"xt{kc}_0")
                    nc.sync.dma_start(xt0[:], xt_d[kc * 128:(kc + 1) * 128, 0:512])
                    xtt[kc][0] = xt0
                rts = [xp.tile([128, 512], bf16, tag=f"rt{t_i}", name=f"rt{t_i}")
                       for t_i in range(NT)]
                for t_i in range(4):
                    nc.sync.dma_start(rts[t_i][:], rope_d[t_i * 128:(t_i + 1) * 128, :])
                for t_i in range(4, NT):
                    nc.sync.dma_start(rts[t_i][:], rope_d[t_i * 128:(t_i + 1) * 128, :])
                for q in range(1, 4):
                    for kc in range(KC):
                        xtq = xp.tile([128, 512], bf16, tag=f"xt{kc}_{q}")
                        nc.sync.dma_start(
                            xtq[:], xt_d[kc * 128:(kc + 1) * 128, 512 * q:512 * (q + 1)])
                        xtt[kc][q] = xtq
                wos = []
                for kc4 in range(4):
                    wo = wp.tile([128, T], bf16, tag=f"wo{kc4}")
                    nc.sync.dma_start(wo[:], wo_d[kc4 * 128:(kc4 + 1) * 128, :])
                    wos.append(wo)

                # PE warm-up chain: keeps HAM busy during the DMA gate
                psW = psWp.tile([128, 128], fp32)
                for _ in range(NWARM):
                    nc.tensor.matmul(psW[:], iden[:], iden[:], start=True,
                                     stop=True, skip_group_check=True)

                # ones columns of the V tiles (once)
                for j in range(NT):
                    nc.gpsimd.memset(Vs[j][:, 64:128], 1.0)
                    nc.gpsimd.memset(Vs[j][:, 192:256], 1.0)

                pend = []   # pipelined transposes: (Qsb, Ksb, t-slice)

                def emit_transposes():
                    Qsb_p, Ksb_p, ts_p = pend.pop(0)
                    for qh in range(4):
                        pt = pst.tile([128, 128], bf16, tag="pt", name="pt")
                        nc.tensor.transpose(pt[:], Qsb_p[:, qh * 128:(qh + 1) * 128], iden[:])
                        nc.vector.tensor_copy(QT[qh][:, ts_p], pt[:])
                    pt2 = pst.tile([128, 128], bf16, tag="pt", name="pt2")
                    nc.tensor.transpose(pt2[:], Ksb_p[:], iden[:])
                    nc.scalar.copy(KTd[0][0:64, ts_p], pt2[0:64, :])
                    nc.gpsimd.tensor_copy(KTd[0][64:128, ts_p], KTd[0][0:64, ts_p])
                    nc.scalar.copy(KTd[1][0:64, ts_p], pt2[64:128, :])
                    nc.gpsimd.tensor_copy(KTd[1][64:128, ts_p], KTd[1][0:64, ts_p])

                for t_i in range(NT):
                    q, tq = divmod(t_i, 4)
                    ts = slice(t_i * 128, (t_i + 1) * 128)
                    tsq = slice(tq * 128, (tq + 1) * 128)
                    psQ = psA.tile([128, 512], fp32, tag="psQ")
                    psKV = psA.tile([128, 256], fp32, tag="psKV")
                    for kc in range(KC):
                        st, sp = kc == 0, kc == KC - 1
                        lhs = xtt[kc][q][:, tsq]
                        nc.tensor.matmul(psQ[:], lhs, wqs[kc][:], start=st, stop=sp)
                        nc.tensor.matmul(psKV[:], lhs, wkvs[kc][:], start=st, stop=sp)

                    rt = rts[t_i]
                    # cast Q/K to bf16 on ScalarE (enables DVE 2x for rope)
                    Qb = tp.tile([128, 512], bf16, tag="Qb")
                    nc.scalar.copy(Qb[:], psQ[:])
                    Kb = tp.tile([128, 128], bf16, tag="Kb")
                    nc.scalar.copy(Kb[:], psKV[:, 0:128])

                    nc.vector.tensor_copy(Vs[t_i][:, 0:64], psKV[:, 128:192])
                    nc.vector.tensor_copy(Vs[t_i][:, 128:192], psKV[:, 192:256])

                    # RoPE Q on DVE (bf16): y1' = y1*c - y2*s ; y2' = y2*c + y1*s
                    Qsb = qkp.tile([128, 512], bf16, tag="Qsb")
                    q1 = hv(Qb[:], 8, 0, 32)
                    q2 = hv(Qb[:], 8, 32, 32)
                    c8v = hv(rt[:, 0:256], 8, 0, 32)
                    s8v = hv(rt[:, 256:512], 8, 0, 32)
                    ta = tp.tile([128, 256], bf16, tag="ta")
                    tb = tp.tile([128, 256], bf16, tag="tb")
                    tav = hv(ta[:], 8, 0, 32)
                    tbv = hv(tb[:], 8, 0, 32)
                    nc.vector.tensor_tensor(out=tav, in0=q1, in1=c8v, op=MUL)
                    nc.vector.tensor_tensor(out=tbv, in0=q2, in1=s8v, op=MUL)
                    nc.vector.tensor_tensor(out=hv(Qsb[:], 8, 0, 32), in0=tav, in1=tbv, op=SUB)
                    tc_ = tp.tile([128, 256], bf16, tag="tc")
                    td_ = tp.tile([128, 256], bf16, tag="td")
                    tcv = hv(tc_[:], 8, 0, 32)
                    tdv = hv(td_[:], 8, 0, 32)
                    nc.vector.tensor_tensor(out=tcv, in0=q2, in1=c8v, op=MUL)
                    nc.vector.tensor_tensor(out=tdv, in0=q1, in1=s8v, op=MUL)
                    nc.vector.tensor_tensor(out=hv(Qsb[:], 8, 32, 32), in0=tcv, in1=tdv, op=ADD)

                    # RoPE K on DVE (bf16)
                    Ksb = qkp.tile([128, 128], bf16, tag="Ksb")
                    k1 = hv(Kb[:], 2, 0, 32)
                    k2 = hv(Kb[:], 2, 32, 32)
                    c2v = hv(rt[:, 0:64], 2, 0, 32)
                    s2v = hv(rt[:, 256:320], 2, 0, 32)
                    ka = tp.tile([128, 64], bf16, tag="ka")
                    kb = tp.tile([128, 64], bf16, tag="kb")
                    kav = hv(ka[:], 2, 0, 32)
                    kbv = hv(kb[:], 2, 0, 32)
                    nc.vector.tensor_tensor(out=kav, in0=k1, in1=c2v, op=MUL)
                    nc.vector.tensor_tensor(out=kbv, in0=k2, in1=s2v, op=MUL)
                    nc.vector.tensor_tensor(out=hv(Ksb[:], 2, 0, 32), in0=kav, in1=kbv, op=SUB)
                    kc_ = tp.tile([128, 64], bf16, tag="kc")
                    kd_ = tp.tile([128, 64], bf16, tag="kd")
                    kcv = hv(kc_[:], 2, 0, 32)
                    kdv = hv(kd_[:], 2, 0, 32)
                    nc.vector.tensor_tensor(out=kcv, in0=k2, in1=c2v, op=MUL)
                    nc.vector.tensor_tensor(out=kdv, in0=k1, in1=s2v, op=MUL)
                    nc.vector.tensor_tensor(out=hv(Ksb[:], 2, 32, 32), in0=kcv, in1=kdv, op=ADD)

                    pend.append((Qsb, Ksb, ts))
                    if len(pend) > 1:
                        emit_transposes()
                    if t_i in (3, 7):
                        # ride out x-block DMA arrival gaps
                        for _ in range(12):
                            nc.tensor.matmul(psW[:], iden[:], iden[:], start=True,
                                             stop=True, skip_group_check=True)
                for _ in range(len(pend)):
                    emit_transposes()

            # ---------------- Stage B+C: attention + output projection --------
            with (
                tc.tile_pool(name="psS", bufs=2, space="PSUM") as psSp,
                tc.tile_pool(name="psO", bufs=1, space="PSUM") as psOp,
                tc.tile_pool(name="psF", bufs=2, space="PSUM") as psFp,
                tc.tile_pool(name="sa", bufs=3) as sap,
                tc.tile_pool(name="fin", bufs=2) as finp,
                tc.tile_pool(name="osb", bufs=2) as osbp,
            ):
                yUs = {}
                Zs = [None] * 4

                recSs = [None] * 4
                recBs = [None] * 4

                def fin_recip(isc, half, rows=slice(0, 128)):
                    if recSs[isc] is None:
                        recSs[isc] = finp.tile([128, 1024], fp32, tag="recS",
                                               name="recS", bufs=2)
                        recBs[isc] = finp.tile([128, 1024], fp32, tag="recB",
                                               name="recB", bufs=2)
                    hs = slice(512 * half, 512 * half + 512)
                    nc.vector.reciprocal(recSs[isc][rows, hs], Zs[isc][rows, hs])
                    # broadcast row 0 of each 32-partition quadrant: every
                    # block's rec row fills its whole quadrant
                    nc.vector.stream_shuffle(recBs[isc][rows, hs], recSs[isc][rows, hs],
                                             mask=[0] * 32)

                def fin_parts(isc, blocks):
                    recB = recBs[isc]
                    for b in blocks:
                        hp, h2 = b // 2, b % 2
                        r, c = 32 * (b % 4), 512 * (b // 4)
                        recC = finp.tile([64, 512], fp32, tag="recC",
                                         name="recC", bufs=2)
                        nc.vector.tensor_copy(recC[0:32, :], recB[r:r + 32, c:c + 512])
                        nc.vector.tensor_copy(recC[32:64, :], recB[r:r + 32, c:c + 512])
                        nc.vector.tensor_tensor(
                            out=yT[hp][64 * h2:64 * h2 + 64,
                                       isc * 512:(isc + 1) * 512],
                            in0=yUs[(isc, hp, h2)][:], in1=recC[:], op=MUL,
                        )

                def attn_block(isc, hp):
                        kv = hp // 2
                        njc = 4 * isc + 4
                        psOT = [psOp.tile([128, 512], fp32, tag=f"psOT{h2}",
                                          name=f"psOT{h2}") for h2 in range(2)]

                        def emit_S(jc):
                            r = jc - 4 * isc
                            col0 = max(0, r * 128)
                            js = slice(jc * 128, (jc + 1) * 128)
                            isl = slice(isc * 512 + col0, (isc + 1) * 512)
                            SA = sap.tile([128, 1024], bf16, tag="SA", name="SA")
                            psS = psSp.tile([128, 1024], fp32, tag="psS", name="psS")
                            for h2 in range(2):
                                prow = slice(64 * h2, 64 * h2 + 64)
                                nc.tensor.matmul(
                                    psS[:, 512 * h2 + col0:512 * h2 + 512],
                                    KTd[kv][prow, js],
                                    QT[hp][prow, isl],
                                    start=True, stop=True,
                                    tile_position=(64 * h2, 0),
                                )
                            if r < 0:
                                nc.scalar.activation(SA[:], psS[:], EXP, scale=0.125)
                            else:
                                for h2 in range(2):
                                    c = 512 * h2 + col0
                                    nc.scalar.activation(
                                        SA[:, c:512 * h2 + 512], psS[:, c:512 * h2 + 512],
                                        EXP, scale=0.125,
                                    )
                                    nc.vector.tensor_tensor(
                                        out=SA[:, c:c + 128], in0=SA[:, c:c + 128],
                                        in1=mask[:], op=MUL,
                                    )
                            return SA, col0

                        ready = emit_S(0)
                        for jc in range(njc):
                            SA, col0 = ready
                            if jc + 1 < njc:
                                ready = emit_S(jc + 1)
                            for h2 in range(2):
                                nc.tensor.matmul(
                                    psOT[h2][0:65, col0:512],
                                    Vs[jc][:, 128 * kv:128 * kv + 65],
                                    SA[:, 512 * h2 + col0:512 * h2 + 512],
                                    start=(jc == 0), stop=(jc == njc - 1),
                                )
                        # copy-defer: drain psOT fast (DVE y-half, scalar Z-row
                        # into the compact per-isc Z tile); divide happens once
                        # per isc in fin_cluster with a batched reciprocal
                        for h2 in range(2):
                            yU = finp.tile([64, 512], bf16, tag=f"yU{hp}_{h2}",
                                           name="yU", bufs=1)
                            nc.vector.tensor_copy(yU[:], psOT[h2][0:64, :])
                            yUs[(isc, hp, h2)] = yU
                            b = 2 * hp + h2
                            nc.scalar.copy(
                                Zs[isc][32 * (b % 4):32 * (b % 4) + 1,
                                        512 * (b // 4):512 * (b // 4) + 512],
                                psOT[h2][64:65, :])


                def warm_fill(n):
                    # keep HAM warm while the fin chain drains on DVE
                    psD = psFp.tile([128, 512], fp32, tag="psF")
                    for _ in range(n):
                        nc.tensor.matmul(psD[:, 0:128], iden[:], iden[:],
                                         start=True, stop=True,
                                         skip_group_check=True)

                def stage_C(isc, ts_range=None):
                    for t_i in ts_range if ts_range else range(4 * isc, 4 * isc + 4):
                        ts = slice(t_i * 128, (t_i + 1) * 128)
                        for ec in range(4):
                            es = slice(ec * 512, (ec + 1) * 512)
                            psF = psFp.tile([128, 512], fp32, tag="psF")
                            for kc4 in range(4):
                                nc.tensor.matmul(
                                    psF[:], yT[kc4][:, ts], wos[kc4][:, es],
                                    start=(kc4 == 0), stop=(kc4 == 3),
                                )
                            ot = osbp.tile([128, 512], bf16, tag="ot")
                            nc.vector.tensor_copy(ot[:], psF[:])
                            nc.sync.dma_start(out_d[ts, es], ot[:])

                for isc in range(4):
                    Zs[isc] = finp.tile([128, 1024], fp32, tag="Zs", name="Zs", bufs=2)
                    nc.gpsimd.memset(Zs[isc][:], 1.0)
                    for hp in range(4):
                        attn_block(isc, hp)
                        if isc > 0:
                            if hp == 0:
                                fin_recip(isc - 1, 0)
                            elif hp == 1:
                                fin_recip(isc - 1, 1)
                                fin_parts(isc - 1, range(0, 4))
                            elif hp == 2:
                                fin_parts(isc - 1, range(4, 8))
                            else:
                                warm_fill(20)
                                stage_C(isc - 1)
                        if isc == 3:
                            # last block: finalize as eagerly as Z allows
                            if hp == 2:
                                fin_recip(3, 0)
                                fin_parts(3, range(0, 4))
                                fin_recip(3, 1, rows=slice(0, 64))
                                fin_parts(3, range(4, 6))
                fin_recip(3, 1, rows=slice(64, 128))
                fin_parts(3, range(6, 8))
                warm_fill(140)
                stage_C(3)

    nc.compile()
    return nc


def _host_prep(inputs):
    x = np.asarray(inputs["x"], np.float32)
    Wq = np.asarray(inputs["Wq"], np.float32)
    bq = np.asarray(inputs["bq"], np.float32)
    Wk = np.asarray(inputs["Wk"], np.float32)
    bk = np.asarray(inputs["bk"], np.float32)
    Wv = np.asarray(inputs["Wv"], np.float32)
    bv = np.asarray(inputs["bv"], np.float32)
    Wo = np.asarray(inputs["Wo"], np.float32)

    pos = np.arange(1, T + 1, dtype=np.float32)[:, None]
    freqs = 10000.0 ** (-(2.0 * np.arange(D // 2, dtype=np.float32)) / D)
    theta = pos * freqs
    cos_t = np.cos(theta).astype(np.float32)
    sin_t = np.sin(theta).astype(np.float32)
    ropeT = np.ascontiguousarray(np.concatenate(
        [np.tile(cos_t, (1, 8)), np.tile(sin_t, (1, 8))], axis=1)).astype(BF16)
    mask = (np.arange(128)[:, None] <= np.arange(128)[None, :]).astype(BF16)
    iden = np.eye(128, dtype=BF16)

    xT_aug = np.zeros((B, KAUG, T), np.float32)
    for b in range(B):
        xT_aug[b, :E] = x[b].T
        xT_aug[b, E] = 1.0
    xT_aug = xT_aug.astype(BF16)

    in_maps = []
    for c in range(8):
        b, p = c // 4, c % 4
        wq_a = np.zeros((KAUG, 512), np.float32)
        wq_a[:E] = Wq[:, 512 * p:512 * (p + 1)]
        wq_a[E] = bq[512 * p:512 * (p + 1)]
        wk_a = np.zeros((KAUG, 128), np.float32)
        wk_a[:E] = Wk[:, 128 * p:128 * (p + 1)]
        wk_a[E] = bk[128 * p:128 * (p + 1)]
        wv_a = np.zeros((KAUG, 128), np.float32)
        wv_a[:E] = Wv[:, 128 * p:128 * (p + 1)]
        wv_a[E] = bv[128 * p:128 * (p + 1)]
        in_maps.append({
            "xt": xT_aug[b],
            "wq": wq_a.astype(BF16),
            "wkv": np.concatenate([wk_a, wv_a], axis=1).astype(BF16),
            "wo": np.ascontiguousarray(Wo[512 * p:512 * (p + 1), :]).astype(BF16),
            "rope": ropeT,
            "mask": mask, "iden": iden,
        })
    return in_maps


def _run(inputs, trace=False):
    from concourse.bass_utils import run_bass_kernel_spmd

    if "nc" not in _cache:
        _cache["nc"] = _build_program()
    nc = _cache["nc"]
    in_maps = _host_prep(inputs)
    res = run_bass_kernel_spmd(nc, in_maps, core_ids=list(range(8)), trace=trace)
    bo = np.asarray(inputs["bo"], np.float32)
    out = np.zeros((B, T, E), np.float32)
    for b in range(B):
        acc = bo[None, :].repeat(T, 0).astype(np.float32)
        for c in range(4 * b, 4 * b + 4):
            acc = acc + res.results[c]["out"].astype(np.float32)
        out[b] = acc
    return out, res


def kernel(**inputs):
    out, _ = _run(inputs, trace=False)
    return out

